# revision 8
# baseline (speedup 1.0000x reference)
"""CrossViewTransformer kernel for 8 Trainium2 NeuronCores.

Math (per batch element b, n = H*W = 4096):
    q = wq @ xq + bq            [8, n]
    k = wk @ xr + bk            [8, n]
    v = wv @ xr + bv            [64, n]
    energy[j, i] = sum_p k[p, j] q[p, i]
    att = softmax(energy, axis=-1)          (softmax over i)
    z[c, j] = sum_i v[c, i] att[j, i]
    out = xq + z

Device strategy (data-parallel: one batch element per core):
  * The tiny projections (0.15% of FLOPs) are computed on the HOST and
    uploaded in the exact device layout: q/k replicated at partition
    strips 0/32/64/96 (bf16), v^T in 32 blocks of [128, 65] with a ones
    column (bf16), xq fp32 for the residual add. This removes the whole
    on-device setup phase (projection matmuls + fp32->bf16 casts).
  * Energy is computed TRANSPOSED: eT[i, j] = sum_p q[p, i] k[p, j] in
    [128(i) x 1024(j)] PSUM chunks. Softmax over i then needs no vector
    reductions: N = exp(eT) and the denominator s[j] falls out of the z
    matmul via the ones column of v^T (row 64 of the accumulated zu).
    Max-subtraction is skipped: energies are O(1) (|e| < ~5).
  * The 16.8M exp() evaluations per core are the roofline. They are
    SPLIT between ScalarE (exact spline exp, ~1147ns per [128,1024]
    chunk) and VectorE via a custom single-pass DVE op that computes
    (c + b*x + a*x^2)^16 ~= exp(x) in one 8-stage ALU chain
    (quadratic Horner + 4 squarings) at 1 elem/cycle/lane
    (~1192ns/chunk from PSUM). Minimax-fit coefficients give 2.3%
    max elementwise error; through the softmax ratio (numerator and
    denominator share the error field) the end-to-end output error is
    ~5e-5.
  * Energy matmuls (K=8) are packed 4-per-PE-array with tile_position
    row tiling across two consecutive chunks, so the PE keeps up with
    the ACT+DVE exp pace even when the HAM clock gate is cold.
  * PSUM: 6 banks = energy chunks ([128,1024] x 3 rotating), 2 banks =
    z accumulator [65, 1024]; j processed in four 1024-wide blocks so
    per-block finalize overlaps the next block's compute.
  * Finalize per block: 1/s on DVE, PSUM evacuation on DVE, s-broadcast
    via a DRAM bounce DMA, z*r and +xq on GpSimd (keeps the DVE FIFO
    free of DMA-dependent ops -> no head-of-line blocking of exps).
"""

import sys

if "/opt/trn_rl_repo" not in sys.path:
    sys.path.insert(0, "/opt/trn_rl_repo")

from contextlib import ExitStack

import numpy as np
import ml_dtypes

import concourse.tile as tile
from concourse import bacc, mybir
from concourse.bass_utils import run_bass_kernel_spmd

B = 8
C = 64
HW = 4096
PROJ = 8
NCORES = 8

F32 = mybir.dt.float32
BF16 = mybir.dt.bfloat16
EXP = mybir.ActivationFunctionType.Exp

NT = HW // 128  # 32 i-tiles
JBW = 1024  # j block width (z psum = 2 banks)
NJB = HW // JBW  # 4
VTW = C + 1  # 65: v^T block width incl. ones column

# exp(x) ~= (EC + EB*x + EA*x^2)^16, minimax on [-5.2, 5.2]
EA = 0.001940256367717379
EB = 0.06331772889888164
EC = 1.0003434322477127

# chunks (by i-tile index mod 16) handled by the DVE approx-exp op:
# 7/16 of chunks -> DVE, 9/16 -> ScalarE (balances the two engines).
DVE_T = frozenset((1, 3, 5, 7, 9, 11, 13))


def _register_exp16():
    """Register the single-pass approx-exp custom DVE op (documented
    workflow: append a DveOp to dve_ops.OPS; codegen + table-gen both
    read that module-level list within this process)."""
    from concourse import dve_ops as dvo
    from concourse.dve_spec import Spec, Src0, C0, C1, C2, sq, lower, _has_src1
    from concourse.dve_uop import DveOpSpec

    name = "EXP16_ANT"
    if name in dvo._SUB_OPCODE_FOR_NAME:
        return next(op for op in dvo.OPS if op.name == name)

    body = sq(sq(sq(sq(C2 + Src0 * (C1 + Src0 * C0)))))

    def _ref(in0, in1, s0, s1, imm2):
        p = (imm2 + in0 * (s1 + in0 * s0)).astype(np.float32)
        for _ in range(4):
            p = (p * p).astype(np.float32)
        return p

    spec = Spec(body=body, reference=_ref)
    row = dvo._CUSTOM_DVE_ROW_BASE + len(dvo.OPS)
    shas = {}
    for ver in ("v3", "v4"):
        uops = lower(spec, ver=ver)
        shas[ver] = DveOpSpec(
            name=name, opcode=row, uops=uops, rd1_en=_has_src1(spec)
        ).sha(ver)
    op = dvo.DveOp(name, spec, subdim=False, uops_sha=shas)
    dvo.OPS.append(op)
    dvo._SUB_OPCODE_FOR_NAME[name] = row
    dvo.CUSTOM_DVE_SPECS[name] = spec
    return op


def _build_nc():
    exp16 = _register_exp16()
    nc = bacc.Bacc("TRN2", target_bir_lowering=False, debug=False, num_devices=NCORES)

    qrep_d = nc.dram_tensor("qrep", [128, HW], BF16, kind="ExternalInput").ap()
    krep_d = nc.dram_tensor("krep", [128, HW], BF16, kind="ExternalInput").ap()
    vt_d = nc.dram_tensor("vt", [128, NT * VTW], BF16, kind="ExternalInput").ap()
    xq_d = nc.dram_tensor("xq", [C, HW], F32, kind="ExternalInput").ap()
    out_d = nc.dram_tensor("out", [C, HW], F32, kind="ExternalOutput").ap()
    rb_d = nc.dram_tensor("rscratch", [NJB, JBW], F32).ap()

    with tile.TileContext(nc) as tc, ExitStack() as ctx:
        singles = ctx.enter_context(tc.tile_pool(name="singles", bufs=1))

        qrep_sb = singles.tile([128, HW], BF16)
        krep_sb = singles.tile([128, HW], BF16)
        vt_sb = singles.tile([128, NT * VTW], BF16)
        xq_sb = singles.tile([C, HW], F32)
        aw_in = singles.tile([128, 8], F32)
        aw_out = singles.tile([128, 8], BF16)

        # ACT exp-table preload: a dummy activation first in program order
        # so the ~2.7us PSEUDO_LOAD_ACT_FUNC_SET runs during the DMA wait.
        nc.gpsimd.memset(aw_in[:, :], 0.0)
        nc.scalar.activation(out=aw_out[:, :], in_=aw_in[:, :], func=EXP)

        # input DMAs, most-urgent first. Only the first qrep chunk rides the
        # ACT hw queue (a DMA occupies the issuing engine's FIFO for the
        # transfer, and ACT must be free for exp as soon as possible); all
        # bulk prefetch goes on the SP queue in need-order.
        nc.scalar.dma_start(out=qrep_sb[:, 0:JBW], in_=qrep_d[:, 0:JBW])
        nc.sync.dma_start(out=krep_sb[:, 0:JBW], in_=krep_d[:, 0:JBW])
        nc.sync.dma_start(out=vt_sb[:, :], in_=vt_d[:, :])
        for ci in range(1, 4):
            nc.sync.dma_start(
                out=qrep_sb[:, ci * JBW : (ci + 1) * JBW],
                in_=qrep_d[:, ci * JBW : (ci + 1) * JBW],
            )
        for ci in range(1, 4):
            nc.sync.dma_start(
                out=krep_sb[:, ci * JBW : (ci + 1) * JBW],
                in_=krep_d[:, ci * JBW : (ci + 1) * JBW],
            )
        for ci in range(4):
            nc.sync.dma_start(
                out=xq_sb[:, ci * JBW : (ci + 1) * JBW],
                in_=xq_d[:, ci * JBW : (ci + 1) * JBW],
            )

        epool = ctx.enter_context(tc.tile_pool(name="epsum", bufs=3, space="PSUM"))
        zpool = ctx.enter_context(tc.tile_pool(name="zpsum", bufs=1, space="PSUM"))
        ntpool = ctx.enter_context(tc.tile_pool(name="nt", bufs=4))
        fpool = ctx.enter_context(tc.tile_pool(name="fin", bufs=2))

        # HAM warm-up: ~4us of back-to-back matmuls while the input DMAs
        # land. The PE clock gate only releases (1.2 -> 2.4 GHz) after a
        # ~3.4us near-fully-busy window, and the ACT-paced main loop never
        # provides one — measured: without this burst every matmul in the
        # kernel runs cold and the PE becomes the critical path.
        warm_sb = singles.tile([128, 512], BF16)
        nc.vector.memset(warm_sb[:, :], 0.0)
        wp = epool.tile([128, JBW], F32, tag="e")
        for _ in range(16):
            nc.tensor.matmul(
                wp[:, 0:512],
                lhsT=warm_sb[:, 0:128],
                rhs=warm_sb[:, :],
                start=True,
                stop=True,
            )

        for jb in range(NJB):
            j0 = jb * JBW
            zps = zpool.tile([VTW, JBW], F32)
            eps = [None] * NT
            nts = [None] * NT

            def emit_energy(t):
                # [128, 1024] chunk as 2 concurrent K=8 matmuls at row
                # strips; consecutive t use disjoint strip pairs so 4 MMs
                # (2 chunks) stream concurrently through the PE array.
                ep = epool.tile([128, JBW], F32, tag="e")
                eps[t] = ep
                if t % 4 == 0:
                    # activity-density insurance against HAM re-throttle: a
                    # throwaway matmul into the buffer right before its
                    # real refill (~213ns, PE has ~40% slack at warm clock)
                    nc.tensor.matmul(
                        ep[:, 0:512],
                        lhsT=warm_sb[:, 0:128],
                        rhs=warm_sb[:, :],
                        start=True,
                        stop=True,
                    )
                base = 0 if (t % 2 == 0) else 64
                for h in range(2):
                    strip = base + 32 * h
                    jc = j0 + h * 512
                    nc.tensor.matmul(
                        ep[:, h * 512 : (h + 1) * 512],
                        lhsT=qrep_sb[strip : strip + PROJ, t * 128 : (t + 1) * 128],
                        rhs=krep_sb[strip : strip + PROJ, jc : jc + 512],
                        start=True,
                        stop=True,
                        tile_position=(strip, 0),
                    )

            def emit_exp(t):
                nt_t = ntpool.tile([128, JBW], BF16)
                nts[t] = nt_t
                if (t % 16) in DVE_T or (t == 14 and jb < 2):
                    nc.vector._custom_dve(
                        exp16,
                        out=nt_t[:, :],
                        in0=eps[t][:, :],
                        s0=EA,
                        s1=EB,
                        imm2=EC,
                    )
                else:
                    nc.scalar.activation(out=nt_t[:, :], in_=eps[t][:, :], func=EXP)

            def emit_z(t):
                for c4 in range(2):
                    nc.tensor.matmul(
                        zps[:, c4 * 512 : (c4 + 1) * 512],
                        lhsT=vt_sb[:, t * VTW : (t + 1) * VTW],
                        rhs=nts[t][:, c4 * 512 : (c4 + 1) * 512],
                        start=(t == 0),
                        stop=(t == NT - 1),
                    )

            for t in range(NT):
                emit_energy(t)
                emit_exp(t)
                if t > 1:
                    emit_z(t - 2)
            emit_z(NT - 2)
            emit_z(NT - 1)

            # ---- finalize: out = xq + z / s ------------------------------
            # Evacuate zu (incl. its s row) from PSUM on DVE, spread the s
            # row over 128 partitions (SBUF->SBUF DMA), 1/s at full lane
            # occupancy, broadcast r back over partitions via a DRAM
            # bounce (DMA partition-step-0 source is DRAM-only). The
            # DMA-dependent multiply/add run on the otherwise-idle GpSimd
            # so the DVE FIFO never waits on a DMA.
            z_sb = fpool.tile([VTW, JBW], F32, tag="z")
            nc.vector.tensor_copy(out=z_sb[:, :], in_=zps[:, :])
            ss_sb = fpool.tile([128, JBW // 128], F32, tag="ss")
            nc.sync.dma_start(out=ss_sb[:, :], in_=z_sb[C : C + 1, :])
            rr_sb = fpool.tile([128, JBW // 128], F32, tag="rr")
            nc.vector.reciprocal(out=rr_sb[:, :], in_=ss_sb[:, :])
            nc.sync.dma_start(out=rb_d[jb, :], in_=rr_sb[:, :])
            rb_sb = fpool.tile([C, JBW], F32, tag="rb")
            nc.sync.dma_start(
                out=rb_sb[:, :], in_=rb_d[jb : jb + 1, :].partition_broadcast(C)
            )
            o_sb = fpool.tile([C, JBW], F32, tag="o")
            for h in range(2):
                sl = slice(h * 512, (h + 1) * 512)
                nc.gpsimd.tensor_mul(o_sb[:, sl], z_sb[0:C, sl], rb_sb[:, sl])
                nc.gpsimd.tensor_add(
                    o_sb[:, sl], o_sb[:, sl], xq_sb[:, j0 + h * 512 : j0 + (h + 1) * 512]
                )
                nc.sync.dma_start(
                    out=out_d[:, j0 + h * 512 : j0 + (h + 1) * 512], in_=o_sb[:, sl]
                )

    nc.compile()
    return nc


_NC = None


def _get_nc():
    global _NC
    if _NC is None:
        _NC = _build_nc()
    return _NC


def _make_in_maps(query_x, ref_x, wq, bq, wk, bk, wv, bv):
    query_x = np.ascontiguousarray(np.asarray(query_x, dtype=np.float32))
    ref_x = np.ascontiguousarray(np.asarray(ref_x, dtype=np.float32))
    wq = np.asarray(wq, dtype=np.float32)
    bq = np.asarray(bq, dtype=np.float32)
    wk = np.asarray(wk, dtype=np.float32)
    bk = np.asarray(bk, dtype=np.float32)
    wv = np.asarray(wv, dtype=np.float32)
    bv = np.asarray(bv, dtype=np.float32)

    xq = query_x.reshape(B, C, HW)
    xr = ref_x.reshape(B, C, HW)
    # host-side projections (tiny: 0.15% of kernel FLOPs)
    q = np.einsum("oc,bci->boi", wq, xq) + bq[None, :, None]  # [B, 8, HW]
    k = np.einsum("oc,bci->boi", wk, xr) + bk[None, :, None]  # [B, 8, HW]
    v = np.einsum("oc,bci->boi", wv, xr) + bv[None, :, None]  # [B, 64, HW]

    in_maps = []
    for b in range(B):
        qrep = np.zeros((128, HW), dtype=np.float32)
        krep = np.zeros((128, HW), dtype=np.float32)
        for r in range(4):
            qrep[32 * r : 32 * r + PROJ] = q[b]
            krep[32 * r : 32 * r + PROJ] = k[b]
        # v^T blocks: vt[p, t*65 + c] = v[c, t*128 + p]; ones column at c=64
        vt = np.ones((128, NT, VTW), dtype=np.float32)
        vt[:, :, :C] = v[b].reshape(C, NT, 128).transpose(2, 1, 0)
        in_maps.append(
            {
                "qrep": np.ascontiguousarray(qrep.astype(ml_dtypes.bfloat16)),
                "krep": np.ascontiguousarray(krep.astype(ml_dtypes.bfloat16)),
                "vt": np.ascontiguousarray(
                    vt.reshape(128, NT * VTW).astype(ml_dtypes.bfloat16)
                ),
                "xq": np.ascontiguousarray(xq[b]),
            }
        )
    return in_maps


def kernel(query_x, ref_x, wq, bq, wk, bk, wv, bv):
    nc = _get_nc()
    in_maps = _make_in_maps(query_x, ref_x, wq, bq, wk, bk, wv, bv)
    res = run_bass_kernel_spmd(nc, in_maps, core_ids=list(range(NCORES)))
    out = np.stack([r["out"].reshape(C, 64, 64) for r in res.results], axis=0)
    return np.ascontiguousarray(out.astype(np.float32))


# revision 12
# speedup vs baseline: 1.0417x; 1.0417x over previous
"""CrossViewTransformer kernel for 8 Trainium2 NeuronCores.

Math (per batch element b, n = H*W = 4096):
    q = wq @ xq + bq            [8, n]
    k = wk @ xr + bk            [8, n]
    v = wv @ xr + bv            [64, n]
    energy[j, i] = sum_p k[p, j] q[p, i]
    att = softmax(energy, axis=-1)          (softmax over i)
    z[c, j] = sum_i v[c, i] att[j, i]
    out = xq + z

Device strategy (data-parallel: one batch element per core):
  * The tiny projections (0.15% of FLOPs) are computed on the HOST and
    uploaded in the exact device layout: q/k replicated at partition
    strips 0/32/64/96 (bf16), v^T in 32 blocks of [128, 65] with a ones
    column (bf16), xq fp32 for the residual add. This removes the whole
    on-device setup phase (projection matmuls + fp32->bf16 casts).
  * Energy is computed TRANSPOSED: eT[i, j] = sum_p q[p, i] k[p, j] in
    [128(i) x 1024(j)] PSUM chunks. Softmax over i then needs no vector
    reductions: N = exp(eT) and the denominator s[j] falls out of the z
    matmul via the ones column of v^T (row 64 of the accumulated zu).
    Max-subtraction is skipped: energies are O(1) (|e| < ~5).
  * The 16.8M exp() evaluations per core are the roofline. They are
    SPLIT between ScalarE (exact spline exp, ~1147ns per [128,1024]
    chunk) and VectorE via a custom single-pass DVE op that computes
    (c + b*x + a*x^2)^16 ~= exp(x) in one 8-stage ALU chain
    (quadratic Horner + 4 squarings) at 1 elem/cycle/lane
    (~1192ns/chunk from PSUM). Minimax-fit coefficients give 2.3%
    max elementwise error; through the softmax ratio (numerator and
    denominator share the error field) the end-to-end output error is
    ~5e-5.
  * Energy matmuls (K=8) are packed 4-per-PE-array with tile_position
    row tiling across two consecutive chunks, so the PE keeps up with
    the ACT+DVE exp pace even when the HAM clock gate is cold.
  * PSUM: 6 banks = energy chunks ([128,1024] x 3 rotating), 2 banks =
    z accumulator [65, 1024]; j processed in four 1024-wide blocks so
    per-block finalize overlaps the next block's compute.
  * Finalize per block: 1/s on DVE, PSUM evacuation on DVE, s-broadcast
    via a DRAM bounce DMA, z*r and +xq on GpSimd (keeps the DVE FIFO
    free of DMA-dependent ops -> no head-of-line blocking of exps).
"""

import sys

if "/opt/trn_rl_repo" not in sys.path:
    sys.path.insert(0, "/opt/trn_rl_repo")

from contextlib import ExitStack

import numpy as np
import ml_dtypes

import concourse.tile as tile
from concourse import bacc, mybir
from concourse.bass_utils import run_bass_kernel_spmd

B = 8
C = 64
HW = 4096
PROJ = 8
NCORES = 8

F32 = mybir.dt.float32
BF16 = mybir.dt.bfloat16
EXP = mybir.ActivationFunctionType.Exp

NT = HW // 128  # 32 i-tiles
JBW = 1024  # j block width (z psum = 2 banks)
NJB = HW // JBW  # 4
VTW = C + 1  # 65: v^T block width incl. ones column

# exp(x) ~= (EC + EB*x + EA*x^2)^16, minimax on [-5.2, 5.2]
EA = 0.001940256367717379
EB = 0.06331772889888164
EC = 1.0003434322477127

# chunks (by i-tile index mod 16) handled by the DVE approx-exp op:
# 7/16 of chunks -> DVE, 9/16 -> ScalarE (balances the two engines).
DVE_T = frozenset((1, 3, 5, 7, 9, 11, 13))


def _register_exp16():
    """Register the single-pass approx-exp custom DVE op (documented
    workflow: append a DveOp to dve_ops.OPS; codegen + table-gen both
    read that module-level list within this process)."""
    from concourse import dve_ops as dvo
    from concourse.dve_spec import Spec, Src0, C0, C1, C2, sq, lower, _has_src1
    from concourse.dve_uop import DveOpSpec

    name = "EXP16_ANT"
    if name in dvo._SUB_OPCODE_FOR_NAME:
        return next(op for op in dvo.OPS if op.name == name)

    body = sq(sq(sq(sq(C2 + Src0 * (C1 + Src0 * C0)))))

    def _ref(in0, in1, s0, s1, imm2):
        p = (imm2 + in0 * (s1 + in0 * s0)).astype(np.float32)
        for _ in range(4):
            p = (p * p).astype(np.float32)
        return p

    spec = Spec(body=body, reference=_ref)
    row = dvo._CUSTOM_DVE_ROW_BASE + len(dvo.OPS)
    shas = {}
    for ver in ("v3", "v4"):
        uops = lower(spec, ver=ver)
        shas[ver] = DveOpSpec(
            name=name, opcode=row, uops=uops, rd1_en=_has_src1(spec)
        ).sha(ver)
    op = dvo.DveOp(name, spec, subdim=False, uops_sha=shas)
    dvo.OPS.append(op)
    dvo._SUB_OPCODE_FOR_NAME[name] = row
    dvo.CUSTOM_DVE_SPECS[name] = spec
    return op


def _build_nc():
    exp16 = _register_exp16()
    nc = bacc.Bacc("TRN2", target_bir_lowering=False, debug=False, num_devices=NCORES)

    qrep_d = nc.dram_tensor("qrep", [128, HW], BF16, kind="ExternalInput").ap()
    krep_d = nc.dram_tensor("krep", [128, HW], BF16, kind="ExternalInput").ap()
    vt_d = nc.dram_tensor("vt", [128, NT * VTW], BF16, kind="ExternalInput").ap()
    xq_d = nc.dram_tensor("xq", [C, HW], F32, kind="ExternalInput").ap()
    out_d = nc.dram_tensor("out", [C, HW], F32, kind="ExternalOutput").ap()
    rb_d = nc.dram_tensor("rscratch", [NJB, JBW], F32).ap()

    with tile.TileContext(nc) as tc, ExitStack() as ctx:
        singles = ctx.enter_context(tc.tile_pool(name="singles", bufs=1))

        qrep_sb = singles.tile([128, HW], BF16)
        krep_sb = singles.tile([128, HW], BF16)
        vt_sb = singles.tile([128, NT * VTW], BF16)
        xq_sb = singles.tile([C, HW], F32)
        aw_in = singles.tile([128, 8], F32)
        aw_out = singles.tile([128, 8], BF16)

        # ACT exp-table preload: a dummy activation first in program order
        # so the ~2.7us PSEUDO_LOAD_ACT_FUNC_SET runs during the DMA wait.
        nc.gpsimd.memset(aw_in[:, :], 0.0)
        nc.scalar.activation(out=aw_out[:, :], in_=aw_in[:, :], func=EXP)

        # input DMAs, most-urgent first. Only the first qrep chunk rides the
        # ACT hw queue (a DMA occupies the issuing engine's FIFO for the
        # transfer, and ACT must be free for exp as soon as possible); all
        # bulk prefetch goes on the SP queue in need-order.
        nc.scalar.dma_start(out=qrep_sb[:, 0:JBW], in_=qrep_d[:, 0:JBW])
        nc.sync.dma_start(out=krep_sb[:, 0:JBW], in_=krep_d[:, 0:JBW])
        nc.sync.dma_start(out=vt_sb[:, :], in_=vt_d[:, :])
        for ci in range(1, 4):
            nc.sync.dma_start(
                out=qrep_sb[:, ci * JBW : (ci + 1) * JBW],
                in_=qrep_d[:, ci * JBW : (ci + 1) * JBW],
            )
        for ci in range(1, 4):
            nc.sync.dma_start(
                out=krep_sb[:, ci * JBW : (ci + 1) * JBW],
                in_=krep_d[:, ci * JBW : (ci + 1) * JBW],
            )
        for ci in range(4):
            nc.sync.dma_start(
                out=xq_sb[:, ci * JBW : (ci + 1) * JBW],
                in_=xq_d[:, ci * JBW : (ci + 1) * JBW],
            )

        epool = ctx.enter_context(tc.tile_pool(name="epsum", bufs=3, space="PSUM"))
        zpool = ctx.enter_context(tc.tile_pool(name="zpsum", bufs=1, space="PSUM"))
        ntpool = ctx.enter_context(tc.tile_pool(name="nt", bufs=6))
        fpool = ctx.enter_context(tc.tile_pool(name="fin", bufs=2))

        # HAM warm-up: ~4us of back-to-back matmuls while the input DMAs
        # land. The PE clock gate only releases (1.2 -> 2.4 GHz) after a
        # ~3.4us near-fully-busy window, and the ACT-paced main loop never
        # provides one — measured: without this burst every matmul in the
        # kernel runs cold and the PE becomes the critical path.
        warm_sb = singles.tile([128, 512], BF16)
        nc.vector.memset(warm_sb[:, :], 0.0)
        wp = epool.tile([128, JBW], F32, tag="e")
        for _ in range(16):
            nc.tensor.matmul(
                wp[:, 0:512],
                lhsT=warm_sb[:, 0:128],
                rhs=warm_sb[:, :],
                start=True,
                stop=True,
            )

        for jb in range(NJB):
            j0 = jb * JBW
            zps = zpool.tile([VTW, JBW], F32)
            eps = [None] * NT
            nts = [None] * NT

            def emit_energy(t):
                # [128, 1024] chunk as 2 concurrent K=8 matmuls at row
                # strips; consecutive t use disjoint strip pairs so 4 MMs
                # (2 chunks) stream concurrently through the PE array.
                ep = epool.tile([128, JBW], F32, tag="e")
                eps[t] = ep
                if t % 2 == 0:
                    # activity-density insurance against HAM re-throttle: a
                    # throwaway matmul into the buffer right before its
                    # real refill (~213ns, PE has ~40% slack at warm clock)
                    nc.tensor.matmul(
                        ep[:, 0:512],
                        lhsT=warm_sb[:, 0:128],
                        rhs=warm_sb[:, :],
                        start=True,
                        stop=True,
                    )
                base = 0 if (t % 2 == 0) else 64
                for h in range(2):
                    strip = base + 32 * h
                    jc = j0 + h * 512
                    nc.tensor.matmul(
                        ep[:, h * 512 : (h + 1) * 512],
                        lhsT=qrep_sb[strip : strip + PROJ, t * 128 : (t + 1) * 128],
                        rhs=krep_sb[strip : strip + PROJ, jc : jc + 512],
                        start=True,
                        stop=True,
                        tile_position=(strip, 0),
                    )

            def emit_exp(t):
                nt_t = ntpool.tile([128, JBW], BF16)
                nts[t] = nt_t
                if (t % 16) in DVE_T:
                    nc.vector._custom_dve(
                        exp16,
                        out=nt_t[:, :],
                        in0=eps[t][:, :],
                        s0=EA,
                        s1=EB,
                        imm2=EC,
                    )
                else:
                    nc.scalar.activation(out=nt_t[:, :], in_=eps[t][:, :], func=EXP)

            def emit_z(t):
                for c4 in range(2):
                    nc.tensor.matmul(
                        zps[:, c4 * 512 : (c4 + 1) * 512],
                        lhsT=vt_sb[:, t * VTW : (t + 1) * VTW],
                        rhs=nts[t][:, c4 * 512 : (c4 + 1) * 512],
                        start=(t == 0),
                        stop=(t == NT - 1),
                    )

            # z lags exp by 4 chunks: with lag 2 the z matmuls reach the
            # head of the in-order PE FIFO ~1us before their exp input is
            # ready, and that wait blocks the already-runnable next energy
            # pairs behind them (measured: both exp engines 40% idle).
            ZLAG = 4
            for t in range(NT):
                emit_energy(t)
                emit_exp(t)
                if t >= ZLAG:
                    emit_z(t - ZLAG)
            for t in range(NT - ZLAG, NT):
                emit_z(t)

            # ---- finalize: out = xq + z / s ------------------------------
            # Evacuate zu (incl. its s row) from PSUM on DVE, spread the s
            # row over 128 partitions (SBUF->SBUF DMA), 1/s at full lane
            # occupancy, broadcast r back over partitions via a DRAM
            # bounce (DMA partition-step-0 source is DRAM-only). The
            # DMA-dependent multiply/add run on the otherwise-idle GpSimd
            # so the DVE FIFO never waits on a DMA.
            z_sb = fpool.tile([VTW, JBW], F32, tag="z")
            nc.vector.tensor_copy(out=z_sb[:, :], in_=zps[:, :])
            ss_sb = fpool.tile([128, JBW // 128], F32, tag="ss")
            nc.sync.dma_start(out=ss_sb[:, :], in_=z_sb[C : C + 1, :])
            rr_sb = fpool.tile([128, JBW // 128], F32, tag="rr")
            nc.vector.reciprocal(out=rr_sb[:, :], in_=ss_sb[:, :])
            nc.sync.dma_start(out=rb_d[jb, :], in_=rr_sb[:, :])
            rb_sb = fpool.tile([C, JBW], F32, tag="rb")
            nc.sync.dma_start(
                out=rb_sb[:, :], in_=rb_d[jb : jb + 1, :].partition_broadcast(C)
            )
            o_sb = fpool.tile([C, JBW], F32, tag="o")
            for h in range(2):
                sl = slice(h * 512, (h + 1) * 512)
                nc.gpsimd.tensor_mul(o_sb[:, sl], z_sb[0:C, sl], rb_sb[:, sl])
                nc.gpsimd.tensor_add(
                    o_sb[:, sl], o_sb[:, sl], xq_sb[:, j0 + h * 512 : j0 + (h + 1) * 512]
                )
                nc.sync.dma_start(
                    out=out_d[:, j0 + h * 512 : j0 + (h + 1) * 512], in_=o_sb[:, sl]
                )

    nc.compile()
    return nc


_NC = None


def _get_nc():
    global _NC
    if _NC is None:
        _NC = _build_nc()
    return _NC


def _make_in_maps(query_x, ref_x, wq, bq, wk, bk, wv, bv):
    query_x = np.ascontiguousarray(np.asarray(query_x, dtype=np.float32))
    ref_x = np.ascontiguousarray(np.asarray(ref_x, dtype=np.float32))
    wq = np.asarray(wq, dtype=np.float32)
    bq = np.asarray(bq, dtype=np.float32)
    wk = np.asarray(wk, dtype=np.float32)
    bk = np.asarray(bk, dtype=np.float32)
    wv = np.asarray(wv, dtype=np.float32)
    bv = np.asarray(bv, dtype=np.float32)

    xq = query_x.reshape(B, C, HW)
    xr = ref_x.reshape(B, C, HW)
    # host-side projections (tiny: 0.15% of kernel FLOPs)
    q = np.einsum("oc,bci->boi", wq, xq) + bq[None, :, None]  # [B, 8, HW]
    k = np.einsum("oc,bci->boi", wk, xr) + bk[None, :, None]  # [B, 8, HW]
    v = np.einsum("oc,bci->boi", wv, xr) + bv[None, :, None]  # [B, 64, HW]

    in_maps = []
    for b in range(B):
        qrep = np.zeros((128, HW), dtype=np.float32)
        krep = np.zeros((128, HW), dtype=np.float32)
        for r in range(4):
            qrep[32 * r : 32 * r + PROJ] = q[b]
            krep[32 * r : 32 * r + PROJ] = k[b]
        # v^T blocks: vt[p, t*65 + c] = v[c, t*128 + p]; ones column at c=64
        vt = np.ones((128, NT, VTW), dtype=np.float32)
        vt[:, :, :C] = v[b].reshape(C, NT, 128).transpose(2, 1, 0)
        in_maps.append(
            {
                "qrep": np.ascontiguousarray(qrep.astype(ml_dtypes.bfloat16)),
                "krep": np.ascontiguousarray(krep.astype(ml_dtypes.bfloat16)),
                "vt": np.ascontiguousarray(
                    vt.reshape(128, NT * VTW).astype(ml_dtypes.bfloat16)
                ),
                "xq": np.ascontiguousarray(xq[b]),
            }
        )
    return in_maps


def kernel(query_x, ref_x, wq, bq, wk, bk, wv, bv):
    nc = _get_nc()
    in_maps = _make_in_maps(query_x, ref_x, wq, bq, wk, bk, wv, bv)
    res = run_bass_kernel_spmd(nc, in_maps, core_ids=list(range(NCORES)))
    out = np.stack([r["out"].reshape(C, 64, 64) for r in res.results], axis=0)
    return np.ascontiguousarray(out.astype(np.float32))
